# revision 2
# baseline (speedup 1.0000x reference)
"""
Trainium2 kernel for nn_CanonicalLinear (dense_mlp).

Reference computation:
    heads[b, n, c] = x @ W[n].T + b[n]          (8 per-head linears)
    out[b, c]      = sum_n heads[b, n, c] * factor[n]

By linearity this collapses to a single linear layer:
    W_eff[c, d] = sum_n factor[n] * W[n, c, d]
    b_eff[c]    = sum_n factor[n] * b[n, c]
    out         = x @ W_eff.T + b_eff

which is 8x less matmul work than the naive per-head form.

Sharding over the 8 NeuronCores: 2-way data-parallel over the batch
(8192 -> 4096) x 4-way tensor-parallel over num_classes (2048 -> 512).
Core r handles batch half r//4 and class quarter r%4.  The W read for a
c-quarter is additionally split between the two batch-shard peers: each
core loads and factor-reduces HALF its quarter (16MB instead of 32MB)
and the halves are exchanged with a 2-core AllGather, cutting per-core
HBM traffic to x 32MB + W 16MB + gather 6MB + out 8MB = 62MB.

The host supplies each batch shard of x pre-transposed ([D, BS] layout,
a once-per-shard np transpose during sharding) so the contraction dim is
the SBUF partition dim on load and no on-device transposes of x are
needed (on-device PE-transposing x measured 437us vs 249us/iteration).

Per-core device kernel:
  1. DVE reduces W[n, c_half, :] with factor weights -> W_eff half;
     pair AllGather (via DRAM) assembles the full c-quarter W_eff.
  2. PE (tensor engine) transposes W_eff -> W_effT  [d, c]  (fp32 has no
     DMA transpose; transpose-mode matmuls with an identity are used).
  3. Per 4-tile batch block: DMA xT block [128, 16, 512], then per 128-row
     tile accumulate out = xT.T @ W_effT over the 16 contraction chunks in
     PSUM.  Matmuls run in float32r (FP22 reduced precision, 4x faster
     than true fp32 on the PE, rel err ~2e-4 for D=2048 dot products).
  4. The bias (PE-broadcast to all partitions) is added by DVE during
     PSUM->SBUF eviction; DMA out.
"""

import numpy as np

P = 128
B, D, C, N = 8192, 2048, 2048, 8
DP, TP = 2, 4                      # data-parallel x tensor-parallel grid
BS, CS = B // DP, C // TP          # per-core batch rows / out cols
NCORES = DP * TP

_cached_nc = None
W_SPLIT = True
XT_HOST = True
SPLIT_GATHER = True
HALF_REMAP = False
# local->global c-chunk permutation when HALF_REMAP (self-inverse)
CPERM = [0, 2, 1, 3]


def set_grid(dp, tp):
    global DP, TP, BS, CS, GROUPS, _cached_nc
    DP, TP = dp, tp
    BS, CS = B // DP, C // TP
    GROUPS = [[q + i * TP for i in range(DP)] for q in range(TP)]
    _cached_nc = None

# AllGather groups: cores sharing a c-slice (same q, all batch shards)
GROUPS = [[q + i * TP for i in range(DP)] for q in range(TP)]


def _build(bs=BS, cs=CS, d=D, n_heads=N, repeat=1, w_split=False, groups=None, split_deg=DP, xt_host=False, split_gather=False, half_remap=False):
    import concourse.bass as bass
    import concourse.mybir as mybir
    import concourse.tile as tile
    from concourse import bacc
    from concourse.masks import make_identity

    FP32 = mybir.dt.float32
    F32R = mybir.dt.float32r
    MULT = mybir.AluOpType.mult
    ADD = mybir.AluOpType.add

    dk = d // P                    # contraction chunks
    cb = cs // P                   # c chunks per core
    nbt = bs // P                  # batch tiles per core

    cs_in = cs // split_deg if w_split else cs   # per-core W slice width
    cbi = cs_in // P                     # W-reduce c chunks

    nc = bacc.Bacc()
    # with xt_host, the host supplies x already transposed: [d, bs]
    xd = nc.dram_tensor("x", [d, bs] if xt_host else [bs, d], FP32,
                        kind="ExternalInput")
    wd = nc.dram_tensor("w", [n_heads, cs_in, d], FP32, kind="ExternalInput")
    bd = nc.dram_tensor("b", [n_heads, cs], FP32, kind="ExternalInput")
    fd = nc.dram_tensor("f", [n_heads], FP32, kind="ExternalInput")
    od = nc.dram_tensor("out", [bs, cs], FP32, kind="ExternalOutput")
    if w_split:
        # my reduced W_eff half -> AllGather with the batch-pair peer ->
        # full W_eff slice for this c-quarter, in global c order.
        whalf = nc.dram_tensor("whalf", [cs_in, d], FP32)
        if split_gather:
            # one AllGather per 128-c chunk: chunk g's gather/reload/transpose
            # overlaps chunk g+1's load+reduce
            wgathers = [nc.dram_tensor(f"wgather{g}", [split_deg * P, d], FP32)
                        for g in range(cbi)]
        else:
            wgather = nc.dram_tensor("wgather", [cs, d], FP32)

    # keep total SBUF under the ~24.5MB cap: weffT alone is cs*d*4 bytes
    xl_bufs = 4 if cs > 512 else 5
    # xt_host blocks are 4x bigger (4 b-tiles each) -> fewer bufs
    xt_bufs = 3 if xt_host else (5 if cs > 512 else 6)
    with tile.TileContext(nc) as tc:
        with (
            tc.tile_pool(name="singles", bufs=1) as singles,
            tc.tile_pool(name="wload", bufs=4) as wload,
            tc.tile_pool(name="waccp", bufs=2) as waccp,
            tc.tile_pool(name="xload", bufs=xl_bufs) as xload,
            tc.tile_pool(name="xtp", bufs=xt_bufs) as xtp,
            tc.tile_pool(name="outp", bufs=3) as outp,
            tc.tile_pool(name="pst", bufs=3, space="PSUM") as pst,
            tc.tile_pool(name="psw", bufs=2, space="PSUM") as psw,
            tc.tile_pool(name="pso", bufs=5, space="PSUM") as pso,
        ):
            # --- constants ---------------------------------------------
            ident32 = singles.tile([P, P], FP32)
            make_identity(nc, ident32)
            ident_r = singles.tile([P, P], F32R)
            nc.vector.tensor_copy(ident_r, ident32)

            # factor broadcast to all 128 partitions: [P, N]
            f_ap = fd[:]
            f_rep = singles.tile([P, n_heads], FP32)
            nc.gpsimd.dma_start(
                f_rep,
                bass.AP(tensor=f_ap.tensor, offset=f_ap.offset,
                        ap=[[0, P]] + list(f_ap.ap)),
            )

            # DVE copy absorbs the broadcast-DMA waits so the following
            # TensorScalar ops (single ISA wait slot) only ever wait on one
            # semaphore.
            f_use = singles.tile([P, n_heads], FP32)
            nc.vector.tensor_copy(f_use, f_rep)

            # Touch column: tiny DVE copies that absorb DMA-completion
            # semaphore waits, because TensorScalar ops have a single ISA
            # wait slot.
            touch = singles.tile([P, 48], FP32)
            touch_g = singles.tile([P, 48], FP32)

            # b_eff[c] = sum_n f[n] * b[n, c] on the PE (K=8 matmul), then
            # broadcast to all 128 partitions (K=1 matmul with a ones row).
            b_sb = singles.tile([n_heads, cs], FP32)
            nc.sync.dma_start(b_sb, bd[:])
            f8 = singles.tile([n_heads, 1], FP32)
            nc.sync.dma_start(
                f8,
                bass.AP(tensor=f_ap.tensor, offset=f_ap.offset,
                        ap=list(f_ap.ap) + [[1, 1]]),
            )
            ones1 = singles.tile([1, P], FP32)
            nc.vector.memset(ones1, 1.0)
            beff_row = singles.tile([1, cs], FP32)
            for h in range(0, cs, 512):
                hw_ = min(512, cs - h)
                pw = psw.tile([1, 512], FP32, tag="pw")
                nc.tensor.matmul(pw[:, :hw_], f8, b_sb[:, h:h + hw_])
                nc.any.tensor_copy(beff_row[:, h:h + hw_], pw[:, :hw_])
            beff = singles.tile([P, cs], FP32)
            for h in range(0, cs, 512):
                hw_ = min(512, cs - h)
                pw = psw.tile([P, 512], FP32, tag="pw")
                nc.tensor.matmul(pw[:, :hw_], ones1, beff_row[:1, h:h + hw_])
                nc.any.tensor_copy(beff[:, h:h + hw_], pw[:, :hw_])

            for _rep in range(repeat):
                # --- W phase: weighted reduce over heads, then transpose ----
                # weffT[dp, k, c] = W_eff[c, k*P + dp]
                weffT = singles.tile([P, dk, cs], F32R)

                def transpose_chunk(wacc, j):
                    for g in range(dk // 4):
                        pw = psw.tile([P, 4, P], F32R, tag="pw")
                        for u in range(4):
                            k = 4 * g + u
                            nc.tensor.matmul(
                                pw[:, u, :],
                                wacc[:, k * P:(k + 1) * P],
                                ident_r,
                                is_transpose=True,
                            )
                        nc.any.tensor_copy(
                            weffT[:, 4 * g:4 * g + 4, j * P:(j + 1) * P], pw)

                def load_transpose_x(i):
                    xtile = xload.tile([P, d], F32R)
                    nc.sync.dma_start(xtile,
                                      xd[i * P:(i + 1) * P, :].bitcast(F32R))
                    xt = xtp.tile([P, dk, P], F32R)
                    for g in range(dk // 4):
                        pt = pst.tile([P, 4, P], F32R)
                        for u in range(4):
                            k = 4 * g + u
                            nc.tensor.matmul(
                                pt[:, u, :],
                                xtile[:, k * P:(k + 1) * P],
                                ident_r,
                                is_transpose=True,
                            )
                        nc.any.tensor_copy(xt[:, 4 * g:4 * g + 4, :], pt)
                    return xt

                # prefetch + transpose the first x tiles so the PE has work
                # while the W phase streams (no PE work needed when the host
                # pre-transposes x; the pool bufs prefetch DMA instead)
                n_pref = 0 if xt_host else min(4, nbt)
                xt_pref = {}
                for i in range(n_pref):
                    xt_pref[i] = load_transpose_x(i)

                for j in range(cbi):
                    eng = nc.vector
                    tch = touch
                    wacc = waccp.tile([P, d], F32R)
                    for n in range(n_heads):
                        wt = wload.tile([P, d], FP32)
                        nc.sync.dma_start(wt, wd[n, j * P:(j + 1) * P, :])
                        eng.tensor_copy(
                            tch[:, (8 * j + n) % 40:(8 * j + n) % 40 + 1],
                            wt[:, 0:1])
                        if n == 0:
                            eng.tensor_scalar(wacc, wt, f_use[:, 0:1],
                                              None, MULT)
                        else:
                            eng.scalar_tensor_tensor(
                                wacc, wt, f_use[:, n:n + 1], wacc, MULT, ADD)
                    if w_split:
                        # ship my reduced chunk out for the pair AllGather
                        nc.sync.dma_start(
                            whalf[j * P:(j + 1) * P, :].bitcast(F32R), wacc)
                        if split_gather:
                            nc.gpsimd.collective_compute(
                                "AllGather",
                                mybir.AluOpType.bypass,
                                replica_groups=groups,
                                ins=[whalf[j * P:(j + 1) * P, :]],
                                outs=[wgathers[j][:]],
                            )
                            # member m's chunk j is global c-chunk m*cbi+j;
                            # with half_remap it lands at local slot
                            # j*split_deg+m so gather j fills a contiguous
                            # half of weffT
                            for m in range(split_deg):
                                wacc2 = waccp.tile([P, d], F32R)
                                nc.sync.dma_start(
                                    wacc2,
                                    wgathers[j][m * P:(m + 1) * P, :]
                                    .bitcast(F32R))
                                lpos = (j * split_deg + m) if half_remap \
                                    else (m * cbi + j)
                                transpose_chunk(wacc2, lpos)
                    else:
                        transpose_chunk(wacc, j)

                if w_split and not split_gather:
                    nc.gpsimd.collective_compute(
                        "AllGather",
                        mybir.AluOpType.bypass,
                        replica_groups=groups,
                        ins=[whalf[:]],
                        outs=[wgather[:]],
                    )
                    # reload the gathered full slice and transpose it
                    for j in range(cb):
                        wacc = waccp.tile([P, d], F32R)
                        nc.sync.dma_start(
                            wacc, wgather[j * P:(j + 1) * P, :].bitcast(F32R))
                        transpose_chunk(wacc, j)

                # --- main loop over 128-row x tiles -------------------------
                if xt_host:
                    # x arrives pre-transposed [d, bs]: load 4-tile b-blocks
                    # [128, dk, 512] directly -- no PE transposes needed.
                    BLK = 4
                    for blk in range((nbt + BLK - 1) // BLK):
                        nt = min(BLK, nbt - blk * BLK)
                        xtb = xtp.tile([P, dk, BLK * P], F32R, tag="xtb")
                        for k in range(dk):
                            nc.sync.dma_start(
                                xtb[:, k, :nt * P],
                                xd[k * P:(k + 1) * P,
                                   blk * BLK * P:blk * BLK * P + nt * P].bitcast(F32R))
                        ch = 256 if half_remap else 512
                        for u in range(nt):
                            i = blk * BLK + u
                            osb = outp.tile([P, cs], FP32)
                            for h in range(0, cs, ch):
                                hw_ = min(ch, cs - h)
                                po = pso.tile([P, ch], FP32, tag="po")
                                for k in range(dk):
                                    nc.tensor.matmul(
                                        po[:, :hw_],
                                        xtb[:, k, u * P:(u + 1) * P],
                                        weffT[:, k, h:h + hw_],
                                        start=(k == 0),
                                        stop=(k == dk - 1),
                                    )
                                nc.vector.tensor_add(osb[:, h:h + hw_],
                                                     po[:, :hw_],
                                                     beff[:, h:h + hw_])
                            nc.sync.dma_start(od[i * P:(i + 1) * P, :], osb)
                else:
                    for i in range(nbt):
                        xt = xt_pref.pop(i) if i in xt_pref else load_transpose_x(i)

                        osb = outp.tile([P, cs], FP32)
                        for h in range(0, cs, 512):
                            hw_ = min(512, cs - h)
                            po = pso.tile([P, 512], FP32)
                            for k in range(dk):
                                nc.tensor.matmul(
                                    po[:, :hw_],
                                    xt[:, k, :],
                                    weffT[:, k, h:h + hw_],
                                    start=(k == 0),
                                    stop=(k == dk - 1),
                                )
                            nc.vector.tensor_add(osb[:, h:h + hw_], po[:, :hw_],
                                                 beff[:, h:h + hw_])
                        nc.sync.dma_start(od[i * P:(i + 1) * P, :], osb)

    nc.finalize()
    return nc


def _get_nc():
    global _cached_nc
    if _cached_nc is None:
        _cached_nc = _build(bs=BS, cs=CS, w_split=W_SPLIT, groups=GROUPS,
                            split_deg=DP, xt_host=XT_HOST,
                            split_gather=SPLIT_GATHER,
                            half_remap=HALF_REMAP)
    return _cached_nc


def _local_b(bq):
    if not (W_SPLIT and SPLIT_GATHER and HALF_REMAP):
        return np.ascontiguousarray(bq)
    chunks = [bq[:, gc * 128:(gc + 1) * 128] for gc in CPERM]
    return np.ascontiguousarray(np.concatenate(chunks, axis=1))


def _shard_inputs(x, W, b, factor, w_split=W_SPLIT, xt_host=XT_HOST):
    in_maps = []
    cs_in = CS // DP if w_split else CS
    # transpose each batch shard once on the host (layout only; shared by
    # the TP cores of that shard)
    xsh = {}
    for p in range(DP):
        xs = x[p * BS:(p + 1) * BS]
        xsh[p] = np.ascontiguousarray(xs.T) if xt_host else np.ascontiguousarray(xs)
    for r in range(NCORES):
        p, q = divmod(r, TP)
        c0 = q * CS + (p * cs_in if w_split else 0)
        in_maps.append({
            "x": xsh[p],
            "w": np.ascontiguousarray(W[:, c0:c0 + cs_in, :]),
            "b": _local_b(b[:, q * CS:(q + 1) * CS]),
            "f": np.ascontiguousarray(factor),
        })
    return in_maps


def _unshard_into(out, r, oc):
    p, q = divmod(r, TP)
    remap = W_SPLIT and SPLIT_GATHER and HALF_REMAP
    if remap:
        for l, gc in enumerate(CPERM):
            out[p * BS:(p + 1) * BS,
                q * CS + gc * 128:q * CS + (gc + 1) * 128] = \
                oc[:, l * 128:(l + 1) * 128]
    else:
        out[p * BS:(p + 1) * BS, q * CS:(q + 1) * CS] = oc


def kernel(x, W, b, factor, _trace=False):
    from concourse.bass_utils import run_bass_kernel_spmd

    x = np.asarray(x, dtype=np.float32)
    W = np.asarray(W, dtype=np.float32)
    b = np.asarray(b, dtype=np.float32)
    factor = np.asarray(factor, dtype=np.float32)

    nc = _get_nc()
    in_maps = _shard_inputs(x, W, b, factor)
    res = run_bass_kernel_spmd(nc, in_maps, list(range(NCORES)),
                               trace=_trace)

    out = np.empty((B, C), dtype=np.float32)
    remap = W_SPLIT and SPLIT_GATHER and HALF_REMAP
    for r in range(NCORES):
        p, q = divmod(r, TP)
        oc = res.results[r]["out"]
        if remap:
            for l, gc in enumerate(CPERM):
                out[p * BS:(p + 1) * BS,
                    q * CS + gc * 128:q * CS + (gc + 1) * 128] = \
                    oc[:, l * 128:(l + 1) * 128]
        else:
            out[p * BS:(p + 1) * BS, q * CS:(q + 1) * CS] = oc
    if _trace:
        return out, res
    return out



# revision 4
# speedup vs baseline: 5.6959x; 5.6959x over previous
"""
Trainium2 kernel for nn_CanonicalLinear (dense_mlp).

Reference computation:
    heads[b, n, c] = x @ W[n].T + b[n]          (8 per-head linears)
    out[b, c]      = sum_n heads[b, n, c] * factor[n]

By linearity this collapses to a single linear layer:
    W_eff[c, d] = sum_n factor[n] * W[n, c, d]
    b_eff[c]    = sum_n factor[n] * b[n, c]
    out         = x @ W_eff.T + b_eff

which is 8x less matmul work than the naive per-head form.

Sharding over the 8 NeuronCores: 2-way data-parallel over the batch
(8192 -> 4096) x 4-way tensor-parallel over num_classes (2048 -> 512).
Core r handles batch half r//4 and class quarter r%4.  The W read for a
c-quarter is additionally split between the two batch-shard peers: each
core loads and factor-reduces HALF its quarter (16MB instead of 32MB)
and the halves are exchanged with a 2-core AllGather, cutting per-core
HBM traffic to x 32MB + W 16MB + gather 6MB + out 8MB = 62MB.

The host supplies each batch shard of x pre-transposed ([D, BS] layout,
a once-per-shard np transpose during sharding) so the contraction dim is
the SBUF partition dim on load and no on-device transposes of x are
needed (on-device PE-transposing x measured 437us vs 249us/iteration).

Per-core device kernel:
  1. DVE reduces W[n, c_half, :] with factor weights -> W_eff half;
     pair AllGather (via DRAM) assembles the full c-quarter W_eff.
  2. PE (tensor engine) transposes W_eff -> W_effT  [d, c]  (fp32 has no
     DMA transpose; transpose-mode matmuls with an identity are used).
  3. Per 4-tile batch block: DMA xT block [128, 16, 512], then per 128-row
     tile accumulate out = xT.T @ W_effT over the 16 contraction chunks in
     PSUM.  Matmuls run in float32r (FP22 reduced precision, 4x faster
     than true fp32 on the PE, rel err ~2e-4 for D=2048 dot products).
  4. The bias (PE-broadcast to all partitions) is added by DVE during
     PSUM->SBUF eviction; DMA out.
"""

import numpy as np

P = 128
B, D, C, N = 8192, 2048, 2048, 8
DP, TP = 2, 4                      # data-parallel x tensor-parallel grid
BS, CS = B // DP, C // TP          # per-core batch rows / out cols
NCORES = DP * TP

_cached_nc = None
W_SPLIT = True
XT_HOST = True
SPLIT_GATHER = True
HALF_REMAP = False
# local->global c-chunk permutation when HALF_REMAP (self-inverse)
CPERM = [0, 2, 1, 3]


def set_grid(dp, tp):
    global DP, TP, BS, CS, GROUPS, _cached_nc
    DP, TP = dp, tp
    BS, CS = B // DP, C // TP
    GROUPS = [[q + i * TP for i in range(DP)] for q in range(TP)]
    _cached_nc = None

# AllGather groups: cores sharing a c-slice (same q, all batch shards)
GROUPS = [[q + i * TP for i in range(DP)] for q in range(TP)]


def _build(bs=BS, cs=CS, d=D, n_heads=N, repeat=1, w_split=False, groups=None, split_deg=DP, xt_host=False, split_gather=False, half_remap=False):
    import concourse.bass as bass
    import concourse.mybir as mybir
    import concourse.tile as tile
    from concourse import bacc
    from concourse.masks import make_identity

    FP32 = mybir.dt.float32
    F32R = mybir.dt.float32r
    MULT = mybir.AluOpType.mult
    ADD = mybir.AluOpType.add

    dk = d // P                    # contraction chunks
    cb = cs // P                   # c chunks per core
    nbt = bs // P                  # batch tiles per core

    cs_in = cs // split_deg if w_split else cs   # per-core W slice width
    cbi = cs_in // P                     # W-reduce c chunks

    nc = bacc.Bacc()
    # with xt_host, the host supplies x already transposed: [d, bs]
    xd = nc.dram_tensor("x", [d, bs] if xt_host else [bs, d], FP32,
                        kind="ExternalInput")
    wd = nc.dram_tensor("w", [n_heads, cs_in, d], FP32, kind="ExternalInput")
    bd = nc.dram_tensor("b", [n_heads, cs], FP32, kind="ExternalInput")
    fd = nc.dram_tensor("f", [n_heads], FP32, kind="ExternalInput")
    od = nc.dram_tensor("out", [bs, cs], FP32, kind="ExternalOutput")
    if w_split:
        # my reduced W_eff half -> AllGather with the batch-pair peer ->
        # full W_eff slice for this c-quarter, in global c order.
        whalf = nc.dram_tensor("whalf", [cs_in, d], FP32)
        if split_gather:
            # one AllGather per 128-c chunk: chunk g's gather/reload/transpose
            # overlaps chunk g+1's load+reduce
            wgathers = [nc.dram_tensor(f"wgather{g}", [split_deg * P, d], FP32)
                        for g in range(cbi)]
        else:
            wgather = nc.dram_tensor("wgather", [cs, d], FP32)

    # keep total SBUF under the ~24.5MB cap: weffT alone is cs*d*4 bytes
    xl_bufs = 4 if cs > 512 else 5
    # xt_host blocks are 4x bigger (4 b-tiles each) -> fewer bufs
    xt_bufs = 3 if xt_host else (5 if cs > 512 else 6)
    with tile.TileContext(nc) as tc:
        with (
            tc.tile_pool(name="singles", bufs=1) as singles,
            tc.tile_pool(name="wload", bufs=4) as wload,
            tc.tile_pool(name="waccp", bufs=2) as waccp,
            tc.tile_pool(name="xload", bufs=xl_bufs) as xload,
            tc.tile_pool(name="xtp", bufs=xt_bufs) as xtp,
            tc.tile_pool(name="outp", bufs=3) as outp,
            tc.tile_pool(name="pst", bufs=3, space="PSUM") as pst,
            tc.tile_pool(name="psw", bufs=2, space="PSUM") as psw,
            tc.tile_pool(name="pso", bufs=5, space="PSUM") as pso,
        ):
            # --- constants ---------------------------------------------
            ident32 = singles.tile([P, P], FP32)
            make_identity(nc, ident32)
            ident_r = singles.tile([P, P], F32R)
            nc.vector.tensor_copy(ident_r, ident32)

            # factor broadcast to all 128 partitions: [P, N]
            f_ap = fd[:]
            f_rep = singles.tile([P, n_heads], FP32)
            nc.gpsimd.dma_start(
                f_rep,
                bass.AP(tensor=f_ap.tensor, offset=f_ap.offset,
                        ap=[[0, P]] + list(f_ap.ap)),
            )

            # DVE copy absorbs the broadcast-DMA waits so the following
            # TensorScalar ops (single ISA wait slot) only ever wait on one
            # semaphore.
            f_use = singles.tile([P, n_heads], FP32)
            nc.vector.tensor_copy(f_use, f_rep)

            # Touch column: tiny DVE copies that absorb DMA-completion
            # semaphore waits, because TensorScalar ops have a single ISA
            # wait slot.
            touch = singles.tile([P, 48], FP32)
            touch_g = singles.tile([P, 48], FP32)

            # b_eff[c] = sum_n f[n] * b[n, c] on the PE (K=8 matmul), then
            # broadcast to all 128 partitions (K=1 matmul with a ones row).
            b_sb = singles.tile([n_heads, cs], FP32)
            nc.sync.dma_start(b_sb, bd[:])
            f8 = singles.tile([n_heads, 1], FP32)
            nc.sync.dma_start(
                f8,
                bass.AP(tensor=f_ap.tensor, offset=f_ap.offset,
                        ap=list(f_ap.ap) + [[1, 1]]),
            )
            ones1 = singles.tile([1, P], FP32)
            nc.vector.memset(ones1, 1.0)
            beff_row = singles.tile([1, cs], FP32)
            for h in range(0, cs, 512):
                hw_ = min(512, cs - h)
                pw = psw.tile([1, 512], FP32, tag="pw")
                nc.tensor.matmul(pw[:, :hw_], f8, b_sb[:, h:h + hw_])
                nc.any.tensor_copy(beff_row[:, h:h + hw_], pw[:, :hw_])
            beff = singles.tile([P, cs], FP32)
            for h in range(0, cs, 512):
                hw_ = min(512, cs - h)
                pw = psw.tile([P, 512], FP32, tag="pw")
                nc.tensor.matmul(pw[:, :hw_], ones1, beff_row[:1, h:h + hw_])
                nc.any.tensor_copy(beff[:, h:h + hw_], pw[:, :hw_])

            for _rep in range(repeat):
                # --- W phase: weighted reduce over heads, then transpose ----
                # weffT[dp, k, c] = W_eff[c, k*P + dp]
                weffT = singles.tile([P, dk, cs], F32R)

                def transpose_chunk(wacc, j):
                    for g in range(dk // 4):
                        pw = psw.tile([P, 4, P], F32R, tag="pw")
                        for u in range(4):
                            k = 4 * g + u
                            nc.tensor.matmul(
                                pw[:, u, :],
                                wacc[:, k * P:(k + 1) * P],
                                ident_r,
                                is_transpose=True,
                            )
                        nc.any.tensor_copy(
                            weffT[:, 4 * g:4 * g + 4, j * P:(j + 1) * P], pw)

                def load_transpose_x(i):
                    xtile = xload.tile([P, d], F32R)
                    nc.sync.dma_start(xtile,
                                      xd[i * P:(i + 1) * P, :].bitcast(F32R))
                    xt = xtp.tile([P, dk, P], F32R)
                    for g in range(dk // 4):
                        pt = pst.tile([P, 4, P], F32R)
                        for u in range(4):
                            k = 4 * g + u
                            nc.tensor.matmul(
                                pt[:, u, :],
                                xtile[:, k * P:(k + 1) * P],
                                ident_r,
                                is_transpose=True,
                            )
                        nc.any.tensor_copy(xt[:, 4 * g:4 * g + 4, :], pt)
                    return xt

                # prefetch + transpose the first x tiles so the PE has work
                # while the W phase streams (no PE work needed when the host
                # pre-transposes x; the pool bufs prefetch DMA instead)
                n_pref = 0 if xt_host else min(4, nbt)
                xt_pref = {}
                for i in range(n_pref):
                    xt_pref[i] = load_transpose_x(i)

                for j in range(cbi):
                    eng = nc.vector
                    tch = touch
                    wacc = waccp.tile([P, d], F32R)
                    for n in range(n_heads):
                        wt = wload.tile([P, d], FP32)
                        nc.sync.dma_start(wt, wd[n, j * P:(j + 1) * P, :])
                        eng.tensor_copy(
                            tch[:, (8 * j + n) % 40:(8 * j + n) % 40 + 1],
                            wt[:, 0:1])
                        if n == 0:
                            eng.tensor_scalar(wacc, wt, f_use[:, 0:1],
                                              None, MULT)
                        else:
                            eng.scalar_tensor_tensor(
                                wacc, wt, f_use[:, n:n + 1], wacc, MULT, ADD)
                    if w_split:
                        # ship my reduced chunk out for the pair AllGather
                        nc.sync.dma_start(
                            whalf[j * P:(j + 1) * P, :].bitcast(F32R), wacc)
                        if split_gather:
                            nc.gpsimd.collective_compute(
                                "AllGather",
                                mybir.AluOpType.bypass,
                                replica_groups=groups,
                                ins=[whalf[j * P:(j + 1) * P, :]],
                                outs=[wgathers[j][:]],
                            )
                            # member m's chunk j is global c-chunk m*cbi+j;
                            # with half_remap it lands at local slot
                            # j*split_deg+m so gather j fills a contiguous
                            # half of weffT
                            for m in range(split_deg):
                                wacc2 = waccp.tile([P, d], F32R)
                                nc.sync.dma_start(
                                    wacc2,
                                    wgathers[j][m * P:(m + 1) * P, :]
                                    .bitcast(F32R))
                                lpos = (j * split_deg + m) if half_remap \
                                    else (m * cbi + j)
                                transpose_chunk(wacc2, lpos)
                    else:
                        transpose_chunk(wacc, j)

                if w_split and not split_gather:
                    nc.gpsimd.collective_compute(
                        "AllGather",
                        mybir.AluOpType.bypass,
                        replica_groups=groups,
                        ins=[whalf[:]],
                        outs=[wgather[:]],
                    )
                    # reload the gathered full slice and transpose it
                    for j in range(cb):
                        wacc = waccp.tile([P, d], F32R)
                        nc.sync.dma_start(
                            wacc, wgather[j * P:(j + 1) * P, :].bitcast(F32R))
                        transpose_chunk(wacc, j)

                # --- main loop over 128-row x tiles -------------------------
                if xt_host:
                    # x arrives pre-transposed [d, bs]: load 4-tile b-blocks
                    # [128, dk, 512] directly -- no PE transposes needed.
                    BLK = 4
                    for blk in range((nbt + BLK - 1) // BLK):
                        nt = min(BLK, nbt - blk * BLK)
                        xtb = xtp.tile([P, dk, BLK * P], F32R, tag="xtb")
                        for k in range(dk):
                            nc.sync.dma_start(
                                xtb[:, k, :nt * P],
                                xd[k * P:(k + 1) * P,
                                   blk * BLK * P:blk * BLK * P + nt * P].bitcast(F32R))
                        ch = 256 if half_remap else 512
                        for u in range(nt):
                            i = blk * BLK + u
                            osb = outp.tile([P, cs], FP32)
                            for h in range(0, cs, ch):
                                hw_ = min(ch, cs - h)
                                po = pso.tile([P, ch], FP32, tag="po")
                                for k in range(dk):
                                    nc.tensor.matmul(
                                        po[:, :hw_],
                                        xtb[:, k, u * P:(u + 1) * P],
                                        weffT[:, k, h:h + hw_],
                                        start=(k == 0),
                                        stop=(k == dk - 1),
                                    )
                                nc.vector.tensor_add(osb[:, h:h + hw_],
                                                     po[:, :hw_],
                                                     beff[:, h:h + hw_])
                            nc.sync.dma_start(od[i * P:(i + 1) * P, :], osb)
                else:
                    for i in range(nbt):
                        xt = xt_pref.pop(i) if i in xt_pref else load_transpose_x(i)

                        osb = outp.tile([P, cs], FP32)
                        for h in range(0, cs, 512):
                            hw_ = min(512, cs - h)
                            po = pso.tile([P, 512], FP32)
                            for k in range(dk):
                                nc.tensor.matmul(
                                    po[:, :hw_],
                                    xt[:, k, :],
                                    weffT[:, k, h:h + hw_],
                                    start=(k == 0),
                                    stop=(k == dk - 1),
                                )
                            nc.vector.tensor_add(osb[:, h:h + hw_], po[:, :hw_],
                                                 beff[:, h:h + hw_])
                        nc.sync.dma_start(od[i * P:(i + 1) * P, :], osb)

    nc.finalize()
    return nc


def _build_repeat(repeat):
    return _build(bs=BS, cs=CS, w_split=W_SPLIT, groups=GROUPS,
                  split_deg=DP, xt_host=XT_HOST, split_gather=SPLIT_GATHER,
                  half_remap=HALF_REMAP, repeat=repeat)


def _get_nc():
    global _cached_nc
    if _cached_nc is None:
        import os
        _cached_nc = _build(bs=BS, cs=CS, w_split=W_SPLIT, groups=GROUPS,
                            split_deg=DP, xt_host=XT_HOST,
                            split_gather=SPLIT_GATHER,
                            half_remap=HALF_REMAP,
                            repeat=int(os.environ.get("KREPEAT", "1")))
    return _cached_nc


def _local_b(bq):
    if not (W_SPLIT and SPLIT_GATHER and HALF_REMAP):
        return np.ascontiguousarray(bq)
    chunks = [bq[:, gc * 128:(gc + 1) * 128] for gc in CPERM]
    return np.ascontiguousarray(np.concatenate(chunks, axis=1))


def _shard_inputs(x, W, b, factor, w_split=W_SPLIT, xt_host=XT_HOST):
    in_maps = []
    cs_in = CS // DP if w_split else CS
    # transpose each batch shard once on the host (layout only; shared by
    # the TP cores of that shard)
    xsh = {}
    for p in range(DP):
        xs = x[p * BS:(p + 1) * BS]
        xsh[p] = np.ascontiguousarray(xs.T) if xt_host else np.ascontiguousarray(xs)
    for r in range(NCORES):
        p, q = divmod(r, TP)
        c0 = q * CS + (p * cs_in if w_split else 0)
        in_maps.append({
            "x": xsh[p],
            "w": np.ascontiguousarray(W[:, c0:c0 + cs_in, :]),
            "b": _local_b(b[:, q * CS:(q + 1) * CS]),
            "f": np.ascontiguousarray(factor),
        })
    return in_maps


def _unshard_into(out, r, oc):
    p, q = divmod(r, TP)
    remap = W_SPLIT and SPLIT_GATHER and HALF_REMAP
    if remap:
        for l, gc in enumerate(CPERM):
            out[p * BS:(p + 1) * BS,
                q * CS + gc * 128:q * CS + (gc + 1) * 128] = \
                oc[:, l * 128:(l + 1) * 128]
    else:
        out[p * BS:(p + 1) * BS, q * CS:(q + 1) * CS] = oc


def kernel(x, W, b, factor, _trace=False):
    from concourse.bass_utils import run_bass_kernel_spmd

    x = np.asarray(x, dtype=np.float32)
    W = np.asarray(W, dtype=np.float32)
    b = np.asarray(b, dtype=np.float32)
    factor = np.asarray(factor, dtype=np.float32)

    nc = _get_nc()
    in_maps = _shard_inputs(x, W, b, factor)
    res = run_bass_kernel_spmd(nc, in_maps, list(range(NCORES)),
                               trace=_trace)

    out = np.empty((B, C), dtype=np.float32)
    remap = W_SPLIT and SPLIT_GATHER and HALF_REMAP
    for r in range(NCORES):
        p, q = divmod(r, TP)
        oc = res.results[r]["out"]
        if remap:
            for l, gc in enumerate(CPERM):
                out[p * BS:(p + 1) * BS,
                    q * CS + gc * 128:q * CS + (gc + 1) * 128] = \
                    oc[:, l * 128:(l + 1) * 128]
        else:
            out[p * BS:(p + 1) * BS, q * CS:(q + 1) * CS] = oc
    if _trace:
        return out, res
    return out



# revision 7
# speedup vs baseline: 30.4691x; 5.3493x over previous
"""
Trainium2 kernel for nn_CanonicalLinear (dense_mlp).

Math: out = x @ W_eff.T + b_eff with W_eff = sum_n f[n] W[n],
b_eff = sum_n f[n] b[n] (8x FLOP reduction vs per-head form).

Sharding: DP=2 (batch) x TP=4 (classes); core r=(p,q) computes
out[p-half, q-quarter].  Host supplies x as [d, bs] bf16 and W as
[n, d, csi] bf16 (the core's HALF of its class quarter; the other
half comes from the batch-peer via per-group AllGather of the
factor-REDUCED chunks, which are 8x smaller than raw W).

k-MAJOR emission: for each 128-row d-chunk k, the program emits (in
this order) the W-chunk DMA + its 3-engine reduce, the x-row DMAs
for the window blocks, the window-wave matmuls for chunk k, and any
pending PSUM evictions — so every engine queue sees work in the same
order dependencies resolve, and nothing queues behind the whole W
phase.

Window schedule (8 PSUM banks, bias folded in as a rank-1 K=1
matmul so evictions are plain copies):
  k=0..7:  tiles 0-7 accumulate half-chains (paced by arrivals)
  k=8..15: per chunk, two tile-(0-7) partials evict (DVE/Act) and
           two tiles of 8-15 burst their k0-7 half-chains (chunks
           already present -> full PE speed)
  post:    tiles 0-7 then 8-15 second halves (k8-15, full speed),
           final evict adds the partial (DVE) and stores bf16;
           tiles 16-31 run sequential full chains with Act-copy
           evictions.
"""

import os

import numpy as np

P = 128
B, D, C, N = 8192, 2048, 2048, 8
DP, TP = 2, 4
BS, CS = B // DP, C // TP          # 4096, 512
CSI = CS // DP                     # 256: per-core W slice width
NCORES = 8
DK = D // P                        # 16
NBT = BS // P                      # 32
BLK = 4                            # b-tiles per x DMA block
KH = DK // 2                       # 8: k-split point
GE = 4                             # chunks per AllGather
GROUPS = [[q + i * TP for i in range(DP)] for q in range(TP)]
W_SPLIT = False                    # collectives cost ~15us fixed: not worth it

_cached_nc = None


def _build(bs=BS, cs=CS, repeat=1, w_split=None):
    if w_split is None:
        w_split = W_SPLIT
    import concourse.bass as bass
    import concourse.mybir as mybir
    import concourse.tile as tile
    from concourse import bacc

    FP32 = mybir.dt.float32
    BF16 = mybir.dt.bfloat16
    MULT = mybir.AluOpType.mult
    ADD = mybir.AluOpType.add
    ACT_COPY = mybir.ActivationFunctionType.Copy

    nbt = bs // P
    csi = cs // DP if w_split else cs

    nc = bacc.Bacc()
    xd = nc.dram_tensor("x", [D, bs], BF16, kind="ExternalInput")
    wd = nc.dram_tensor("w", [N, D, csi], BF16, kind="ExternalInput")
    bd = nc.dram_tensor("b", [N, cs], FP32, kind="ExternalInput")
    fd = nc.dram_tensor("f", [N], FP32, kind="ExternalInput")
    od = nc.dram_tensor("out", [bs, cs], BF16, kind="ExternalOutput")
    if w_split:
        whalf = nc.dram_tensor("whalf", [D, csi], BF16)
        ngath = DK // GE
        wgathers = [nc.dram_tensor(f"wgather{g}", [DP * GE * P, csi], BF16)
                    for g in range(ngath)]

    with tile.TileContext(nc) as tc:
        with (
            tc.tile_pool(name="singles", bufs=1) as singles,
            tc.tile_pool(name="wload", bufs=3) as wload,
            tc.tile_pool(name="scp", bufs=16) as scp,
            tc.tile_pool(name="acp", bufs=12) as acp,
            tc.tile_pool(name="wkp", bufs=DK) as wkp,
            tc.tile_pool(name="xtp", bufs=6) as xtp,
            tc.tile_pool(name="pab", bufs=16) as pab,
            tc.tile_pool(name="outp", bufs=4) as outp,
            tc.tile_pool(name="pso", bufs=8, space="PSUM") as pso,
        ):
            # --- factor broadcast + per-engine copies -------------------
            f_ap = fd[:]
            f_rep = singles.tile([P, N], FP32)
            nc.gpsimd.dma_start(
                f_rep,
                bass.AP(tensor=f_ap.tensor, offset=f_ap.offset,
                        ap=[[0, P]] + list(f_ap.ap)),
            )
            f_use = singles.tile([P, N], FP32)
            nc.vector.tensor_copy(f_use, f_rep)
            f_use3 = singles.tile([P, N], FP32)
            nc.scalar.copy(f_use3, f_rep)

            # --- b_eff row (K=8 matmul), bf16, + ones column ------------
            b_sb = singles.tile([N, cs], FP32)
            nc.sync.dma_start(b_sb, bd[:])
            f8 = singles.tile([N, 1], FP32)
            nc.sync.dma_start(
                f8,
                bass.AP(tensor=f_ap.tensor, offset=f_ap.offset,
                        ap=list(f_ap.ap) + [[1, 1]]),
            )
            beff_row = singles.tile([1, cs], FP32)
            pw = pso.tile([1, 512], FP32, name="po", tag="po")
            nc.tensor.matmul(pw[:, :cs], f8, b_sb)
            nc.any.tensor_copy(beff_row, pw[:, :cs])
            beff16 = singles.tile([1, cs], BF16)
            nc.vector.tensor_copy(beff16, beff_row)
            ones1b = singles.tile([1, P], BF16)
            nc.vector.memset(ones1b, 1.0)

            for _rep in range(repeat):
                wk_tiles = [None] * DK
                xtb_blocks = {}

                def get_block_tile(blk, xtb_blocks=xtb_blocks):
                    if blk not in xtb_blocks and blk * BLK < nbt:
                        xtb_blocks[blk] = (
                            xtp.tile([P, DK, BLK * P], BF16, name="xtb"),
                            set())
                    return xtb_blocks.get(blk)

                def emit_xblock(blk):
                    # single 3D-AP DMA for a whole [P, DK, BLK*P] block
                    ent = get_block_tile(blk)
                    if ent is None or len(ent[1]) == DK:
                        return
                    xtb, done = ent
                    assert not done, "block partially row-loaded"
                    done.update(range(DK))
                    nt = min(BLK, nbt - blk * BLK)
                    x_ap = xd[:]
                    nc.scalar.dma_start(
                        xtb[:, :, :nt * P],
                        bass.AP(tensor=x_ap.tensor,
                                offset=x_ap.offset + blk * BLK * P,
                                ap=[[bs, P], [P * bs, DK], [1, nt * P]]),
                    )

                def emit_xrow(blk, k):
                    ent = get_block_tile(blk)
                    if ent is None or k in ent[1]:
                        return
                    xtb, done = ent
                    done.add(k)
                    nt = min(BLK, nbt - blk * BLK)
                    nc.scalar.dma_start(
                        xtb[:, k, :nt * P],
                        xd[k * P:(k + 1) * P,
                           blk * BLK * P:blk * BLK * P + nt * P])

                wdma_done = {}

                def emit_wdma(k):
                    if k in wdma_done:
                        return wdma_done[k]
                    wblk = wload.tile([P, N, csi], BF16)
                    w_ap = wd[:]
                    nc.sync.dma_start(
                        wblk,
                        bass.AP(tensor=w_ap.tensor,
                                offset=w_ap.offset + k * P * csi,
                                ap=[[csi, P], [D * csi, N], [1, csi]]),
                    )
                    wdma_done[k] = wblk
                    return wblk

                def emit_wchunk(k):
                    wblk = emit_wdma(k)
                    s = {}
                    for n in (0, 1, 2, 3, 7):      # DVE scalings (4x)
                        s[n] = scp.tile([P, csi], BF16, name="s")
                        nc.vector.tensor_scalar(
                            s[n], wblk[:, n, :], f_use[:, n:n + 1],
                            None, MULT)
                    for n in (4, 5, 6):            # Activation scalings
                        s[n] = scp.tile([P, csi], BF16, name="s")
                        nc.scalar.activation(
                            s[n], wblk[:, n, :], ACT_COPY,
                            scale=f_use3[:, n:n + 1])
                    a01 = acp.tile([P, csi], BF16, name="a")
                    nc.vector.tensor_tensor(a01, s[0], s[1], ADD)
                    a23 = acp.tile([P, csi], BF16, name="a")
                    nc.vector.tensor_tensor(a23, s[2], s[3], ADD)
                    aa = acp.tile([P, csi], BF16, name="a")
                    nc.vector.tensor_tensor(aa, a01, a23, ADD)
                    aa7 = acp.tile([P, csi], BF16, name="a")
                    nc.vector.tensor_tensor(aa7, aa, s[7], ADD)
                    a45 = acp.tile([P, csi], BF16, name="a")
                    nc.gpsimd.tensor_tensor(a45, s[4], s[5], ADD)
                    a456 = acp.tile([P, csi], BF16, name="a")
                    nc.vector.tensor_tensor(a456, a45, s[6], ADD)
                    if w_split:
                        wh = acp.tile([P, csi], BF16, name="a")
                        nc.gpsimd.tensor_tensor(wh, aa7, a456, ADD)
                        nc.sync.dma_start(whalf[k * P:(k + 1) * P, :], wh)
                        if k % GE == GE - 1:
                            g = k // GE
                            nc.gpsimd.collective_compute(
                                "AllGather",
                                mybir.AluOpType.bypass,
                                replica_groups=GROUPS,
                                ins=[whalf[g * GE * P:(g + 1) * GE * P, :]],
                                outs=[wgathers[g][:]],
                            )
                            for j in range(GE):
                                kk = g * GE + j
                                wk = wkp.tile([P, cs], BF16, name="wk")
                                for m in range(DP):
                                    nc.scalar.dma_start(
                                        wk[:, m * csi:(m + 1) * csi],
                                        wgathers[g][(m * GE + j) * P:
                                                    (m * GE + j + 1) * P, :])
                                wk_tiles[kk] = wk
                    else:
                        wk = wkp.tile([P, cs], BF16, name="wk")
                        nc.gpsimd.tensor_tensor(wk, aa7, a456, ADD)
                        wk_tiles[k] = wk

                def bias_mm(po):
                    nc.tensor.matmul(po[:, :], ones1b, beff16[:1, :],
                                     start=True, stop=False)

                def mm(po, i, k, stop):
                    xtb, _ = get_block_tile(i // BLK)
                    u = i % BLK
                    nc.tensor.matmul(
                        po[:, :],
                        xtb[:, k, u * P:(u + 1) * P],
                        wk_tiles[k][:, :],
                        start=False, stop=stop,
                    )

                # ---- window: tiles 0-7 as full-k sessions --------------
                # 8 PSUM banks <=> 8 sessions; each chunk k feeds 8 matmuls
                # as it arrives.  Only x blocks 0-1 (the window tiles) are
                # streamed during the W window; later blocks follow.
                pcur = {}

                def final_evict(i, po):
                    osb = outp.tile([P, cs], BF16)
                    if i % 2 == 0:
                        nc.vector.tensor_copy(osb, po)
                    else:
                        nc.scalar.copy(osb, po)
                    nc.sync.dma_start(od[i * P:(i + 1) * P, :], osb)

                for k in range(3):
                    emit_wdma(k)
                    emit_xrow(0, k)
                    emit_xrow(1, k)
                for k in range(DK):
                    emit_wchunk(k)
                    emit_xrow(0, k)
                    emit_xrow(1, k)
                    if k == 0:
                        for i in range(8):
                            pcur[i] = pso.tile([P, cs], FP32, name="po",
                                               tag="po")
                            bias_mm(pcur[i])
                    for i in range(8):
                        mm(pcur[i], i, k, stop=(k == DK - 1))

                # next x block streams while the window tiles drain
                for k in range(DK):
                    emit_xrow(2, k)
                for i in range(8):
                    final_evict(i, pcur[i])

                # sequential full chains for tiles 8..nbt-1
                for i in range(8, nbt):
                    blk = i // BLK
                    if i % BLK == 0:
                        for k in range(DK):
                            emit_xrow(blk, k)
                            emit_xrow(blk + 1, k)
                    po = pso.tile([P, cs], FP32, name="po", tag="po")
                    bias_mm(po)
                    for k in range(DK):
                        mm(po, i, k, stop=(k == DK - 1))
                    final_evict(i, po)

    nc.finalize()
    return nc


def _build_repeat(repeat):
    return _build(repeat=repeat)


def _get_nc():
    global _cached_nc
    if _cached_nc is None:
        _cached_nc = _build(repeat=int(os.environ.get("KREPEAT", "1")))
    return _cached_nc


def _shard_inputs(x, W, b, factor):
    import ml_dtypes
    BF = ml_dtypes.bfloat16

    xT = np.ascontiguousarray(x.T.astype(BF))                   # [D, B]
    Wt = np.ascontiguousarray(W.transpose(0, 2, 1).astype(BF))  # [N, D, C]
    in_maps = []
    for r in range(NCORES):
        p, q = divmod(r, TP)
        in_maps.append({
            "x": np.ascontiguousarray(xT[:, p * BS:(p + 1) * BS]),
            "w": np.ascontiguousarray(
                Wt[:, :, q * CS + p * CSI:q * CS + (p + 1) * CSI]
                if W_SPLIT else Wt[:, :, q * CS:(q + 1) * CS]),
            "b": np.ascontiguousarray(b[:, q * CS:(q + 1) * CS]),
            "f": np.ascontiguousarray(factor),
        })
    return in_maps


def _unshard_into(out, r, oc):
    p, q = divmod(r, TP)
    out[p * BS:(p + 1) * BS, q * CS:(q + 1) * CS] = \
        np.asarray(oc, dtype=np.float32)


def kernel(x, W, b, factor, _trace=False):
    from concourse.bass_utils import run_bass_kernel_spmd

    x = np.asarray(x, dtype=np.float32)
    W = np.asarray(W, dtype=np.float32)
    b = np.asarray(b, dtype=np.float32)
    factor = np.asarray(factor, dtype=np.float32)

    nc = _get_nc()
    in_maps = _shard_inputs(x, W, b, factor)
    res = run_bass_kernel_spmd(nc, in_maps, list(range(NCORES)),
                               trace=_trace)

    out = np.empty((B, C), dtype=np.float32)
    for r in range(NCORES):
        _unshard_into(out, r, res.results[r]["out"])
    if _trace:
        return out, res
    return out
